# revision 27
# baseline (speedup 1.0000x reference)
"""Label-smoothed KL loss (AIAYN) on 8 Trainium2 NeuronCores.

Per valid position r with label l, p = dec_output row, u = normalized
token_histo, q = (1-EPS)*onehot(l) + EPS*u:

    kl_r = S1 + (q_l*ln(q_l) - f(l)) - [ sum_v (EPS*u_v)*ln(p_v) + (1-EPS)*ln(p_l) ]

with f(v) = EPS*u_v*ln(EPS*u_v), S1 = sum_v f(v).  The device computes the
heavy weighted log-reduction over the 524MB dec_output plus a per-row
gather of p[r, label]; all small-tensor math runs on the host in f64.

Sharding: 8 cores = 4 batches x 2 sequence halves; each core's p-shard is
a contiguous 512x32000 view of dec_output (row 511 is computed twice, the
duplicate is dropped on host).

The kernel is HBM-bound: each core streams its 65.5 MB shard at the
~358 GB/s per-core HBM share.  w = bf16(EPS*u) is replicated across
partitions on the fly with ones^T @ w matmuls into PSUM; Scalar does Ln in
place; Vector does one fused multiply-accumulate-reduce per PSUM chunk.
Inputs are staged onto the devices with a blocking device_put BEFORE the
NEFF launches so the host->HBM upload never overlaps (and never slows)
the kernel's own HBM reads.
"""

import numpy as np

import concourse.bass as bass
import concourse.bacc as bacc
import concourse.tile as tile
from concourse import mybir

EPS = 0.1
PAD = 0
B, T, V = 4, 1024, 32000
R = 512
P = 128
NRT = R // P
N_CORES = 8

_CACHE = {}

# narrow trailing groups shorten the post-last-DMA drain chain
GROUPS = [2048] + [4096] * 7 + [768, 512]
CW = 2048
NCW = 17


def _build_bass():
    f32 = mybir.dt.float32
    bf16 = mybir.dt.bfloat16
    i32 = mybir.dt.int32
    nc = bacc.Bacc("TRN2", target_bir_lowering=False, debug=False)

    p_t = nc.dram_tensor("p", [R, V], f32, kind="ExternalInput")
    whi_t = nc.dram_tensor("whi", [V], bf16, kind="ExternalInput")
    idx_t = nc.dram_tensor("idx", [R, 1], i32, kind="ExternalInput")
    acc_t = nc.dram_tensor("acc", [R, 1], f32, kind="ExternalOutput")
    plab_t = nc.dram_tensor("plab", [R, 1], f32, kind="ExternalOutput")

    p_ap = p_t.ap()
    p_flat = bass.AP(p_t, 0, [[1, R * V], [1, 1]])

    from contextlib import ExitStack

    with tile.TileContext(nc) as tc, ExitStack() as ctx:
        ppool = ctx.enter_context(tc.tile_pool(name="p", bufs=8))
        wpool = ctx.enter_context(tc.tile_pool(name="wstage", bufs=1))
        wppool = ctx.enter_context(tc.tile_pool(name="wpsum", bufs=2, space="PSUM"))
        apool = ctx.enter_context(tc.tile_pool(name="accs", bufs=NRT))
        spool = ctx.enter_context(tc.tile_pool(name="small", bufs=3 * NRT + 1))

        whi = wpool.tile([1, V], bf16, tag="whi")
        nc.sync.dma_start(whi[:], bass.AP(whi_t, 0, [[1, 1], [1, V]]))

        ones = spool.tile([1, P], bf16, tag="ones")
        nc.gpsimd.memset(ones[:], 1.0)

        for rt in range(NRT):
            it = spool.tile([P, 1], i32, tag="it")
            nc.gpsimd.dma_start(it[:], idx_t.ap()[rt * P:(rt + 1) * P, :])
            g = spool.tile([P, 1], f32, tag="g")
            nc.gpsimd.indirect_dma_start(
                out=g[:],
                out_offset=None,
                in_=p_flat,
                in_offset=bass.IndirectOffsetOnAxis(ap=it[:, :1], axis=0),
            )
            nc.gpsimd.dma_start(plab_t.ap()[rt * P:(rt + 1) * P, :], g[:])

        acccs = [apool.tile([P, NCW], f32, tag=f"accc{rt}", name=f"accc{rt}") for rt in range(NRT)]

        c0 = 0
        ci = 0
        for cj, cwp in enumerate(GROUPS):
            ptiles = []
            for rt in range(NRT):
                t = ppool.tile([P, cwp], f32, tag="pt")
                nc.sync.dma_start(t[:], p_ap[rt * P:(rt + 1) * P, c0:c0 + cwp])
                nc.scalar.activation(t[:], t[:], mybir.ActivationFunctionType.Ln)
                ptiles.append(t)
            for sub in range((cwp + CW - 1) // CW):
                s = sub * CW
                w0 = c0 + s
                cww = min(CW, cwp - s)
                wp = wppool.tile([P, CW], f32, tag="wp")
                for j in range(0, cww, 512):
                    n = min(512, cww - j)
                    nc.tensor.matmul(
                        out=wp[:, j:j + n], lhsT=ones[:], rhs=whi[0:1, w0 + j:w0 + j + n],
                        start=True, stop=True,
                    )
                for rt in range(NRT):
                    nc.vector.affine_mul_reduce(
                        out=ptiles[rt][:, s:s + cww],
                        accum_out=acccs[rt][:, ci:ci + 1],
                        in0=ptiles[rt][:, s:s + cww],
                        in1=wp[:, :cww],
                        scale=1.0,
                        bias=0.0,
                    )
                ci += 1
            c0 += cwp
        assert ci == NCW and c0 == V

        for rt in range(NRT):
            accf = spool.tile([P, 1], f32, tag="accf")
            nc.vector.tensor_reduce(
                accf[:], acccs[rt][:], axis=mybir.AxisListType.X, op=mybir.AluOpType.add
            )
            nc.sync.dma_start(acc_t.ap()[rt * P:(rt + 1) * P, :], accf[:])

    nc.finalize()
    return nc


def _get_cached():
    if "nc" not in _CACHE:
        _CACHE["nc"] = _build_bass()
    return _CACHE["nc"]


def _get_exec():
    """Sharded PJRT callable (same lowering as bass2jax.run_bass_via_pjrt)
    plus metadata, built once.  Kept local so inputs can be device_put and
    block_until_ready'd BEFORE the NEFF launches."""
    if "exec" in _CACHE:
        return _CACHE["exec"]
    import jax
    from jax.experimental.shard_map import shard_map
    from jax.sharding import Mesh, PartitionSpec, NamedSharding
    from concourse import bass2jax
    from concourse.bass2jax import _bass_exec_p, partition_id_tensor

    nc = _get_cached()
    bass2jax.install_neuronx_cc_hook()
    assert nc.dbg_addr is None or not nc.dbg_callbacks

    partition_name = nc.partition_id_tensor.name if nc.partition_id_tensor else None
    in_names, out_names, out_avals, zero_shapes = [], [], [], []
    for alloc in nc.m.functions[0].allocations:
        if not isinstance(alloc, mybir.MemoryLocationSet):
            continue
        name = alloc.memorylocations[0].name
        if alloc.kind == "ExternalInput":
            if name != partition_name:
                in_names.append(name)
        elif alloc.kind == "ExternalOutput":
            out_names.append(name)
            shape = tuple(alloc.tensor_shape)
            dtype = mybir.dt.np(alloc.dtype)
            out_avals.append(jax.core.ShapedArray(shape, dtype))
            zero_shapes.append((shape, dtype))
    n_params = len(in_names)
    all_names = list(in_names) + list(out_names)
    if partition_name is not None:
        all_names.append(partition_name)
    donate = tuple(range(n_params, n_params + len(out_names)))

    def _body(*args):
        operands = list(args)
        if partition_name is not None:
            operands.append(partition_id_tensor())
        return tuple(
            _bass_exec_p.bind(
                *operands,
                out_avals=tuple(out_avals),
                in_names=tuple(all_names),
                out_names=tuple(out_names),
                lowering_input_output_aliases=(),
                sim_require_finite=True,
                sim_require_nnan=True,
                nc=nc,
            )
        )

    devices = jax.devices()[:N_CORES]
    mesh = Mesh(np.asarray(devices), ("core",))
    spec = NamedSharding(mesh, PartitionSpec("core"))
    n_io = n_params + len(out_names)
    sharded = jax.jit(
        shard_map(
            _body,
            mesh=mesh,
            in_specs=(PartitionSpec("core"),) * n_io,
            out_specs=(PartitionSpec("core"),) * len(out_names),
            check_rep=False,
        ),
        donate_argnums=donate,
        keep_unused=True,
    )
    _CACHE["exec"] = (sharded, in_names, out_names, out_avals, zero_shapes, spec)
    return _CACHE["exec"]


def _run_spmd(in_maps, trace=False, reuse_staged=False):
    """Stage inputs on all devices (blocking), then launch the NEFF.
    reuse_staged=True re-executes with the input buffers already on device
    (outputs are donated, so their zero buffers are restaged every call)."""
    import jax

    sharded, in_names, out_names, out_avals, zero_shapes, spec = _get_exec()
    if reuse_staged and "dev_in" in _CACHE:
        dev_args = _CACHE["dev_in"]
    else:
        concat_in = [
            np.concatenate([np.asarray(m[name]) for m in in_maps], axis=0)
            for name in in_names
        ]
        dev_args = jax.device_put(concat_in, [spec] * len(concat_in))
        _CACHE["dev_in"] = dev_args
    concat_zeros = [
        np.zeros((N_CORES * s[0],) + tuple(s[1:]), d) for s, d in zero_shapes
    ]
    dev_zero = jax.device_put(concat_zeros, [spec] * len(concat_zeros))
    dev_in = list(dev_args) + list(dev_zero)
    jax.block_until_ready(dev_in)

    if trace:
        res = _run_traced(sharded, dev_in)
    else:
        res = None
        out_arrs = jax.block_until_ready(sharded(*dev_in))

    if res is not None:
        out_arrs, exec_time_ns = res
    outs = [np.asarray(a) for a in out_arrs]
    results = [
        {
            name: outs[i].reshape(N_CORES, *out_avals[i].shape)[c]
            for i, name in enumerate(out_names)
        }
        for c in range(N_CORES)
    ]

    class _R:
        pass

    r = _R()
    r.results = results
    r.exec_time_ns = res[1] if res is not None else None
    return r


def _run_traced(sharded, dev_in):
    """Wrap one launch in the axon NTFF profile hook and extract the
    NEFF execution time (bench/diagnostic path only)."""
    import glob
    import tempfile
    import jax
    from antenv.axon_hooks import get_axon_ntff_profile_hook
    from concourse import bass_utils

    hook = get_axon_ntff_profile_hook()
    neff_dir = tempfile.mkdtemp()
    with hook(neff_dir, [0]):
        out_arrs = jax.block_until_ready(sharded(*dev_in))

    import gauge.profiler

    nc = _get_cached()
    profile = gauge.profiler.Profile(
        profile_path=bass_utils.FishPath(neff_dir),
        kernel_dev_mode=True,
        profile_on_exit=False,
        bass_kernel=nc.m,
        offline_processing=True,
        fname="*_body*",
        metadata={"artifacts_path": neff_dir},
    )
    exec_time_ns = None
    perfetto_results = profile.to_perfetto(model_index=(0,))
    if perfetto_results:
        exec_time_ns = max(r.exec_time_ns for r in perfetto_results if r.exec_time_ns)
    return out_arrs, exec_time_ns


def _shard_views(dec_input, dec_output):
    shards = []
    for core in range(N_CORES):
        b, h = divmod(core, 2)
        if h == 0:
            p_view = dec_output[b, 0:R]
            labels = dec_input[b, 1:R + 1]
            valid = np.ones(R, dtype=bool)
        else:
            p_view = dec_output[b, R - 1:T - 1]
            labels = dec_input[b, R:T]
            valid = np.ones(R, dtype=bool)
            valid[0] = False
        shards.append((p_view, labels, valid))
    return shards


def kernel(dec_input, dec_output, token_histo, trace=False, reuse_staged=False):
    dec_input = np.asarray(dec_input)
    dec_output = np.ascontiguousarray(np.asarray(dec_output, dtype=np.float32))
    token_histo = np.asarray(token_histo, dtype=np.float32)

    labels_all = dec_input.astype(np.int64)

    u64 = token_histo.astype(np.float64)
    u64 = u64 / u64.sum()
    bf16 = mybir.dt.np(mybir.dt.bfloat16)
    w_hi = (EPS * u64).astype(np.float32).astype(bf16)
    f_tab = EPS * u64 * np.log(EPS * u64)
    S1 = f_tab.sum()
    ql = (1.0 - EPS) + EPS * u64
    g_tab = ql * np.log(ql) - f_tab

    shards = _shard_views(labels_all, dec_output)

    in_maps = []
    host_rows = []
    rowidx = np.arange(R, dtype=np.int64)
    for p_view, labels, valid in shards:
        idx = (rowidx * V + labels).astype(np.int32).reshape(R, 1)
        in_maps.append({"p": p_view, "whi": w_hi, "idx": idx})
        mask = valid & (labels != PAD)
        host_rows.append((labels, mask))

    res = _run_spmd(in_maps, trace=trace, reuse_staged=reuse_staged)

    total = 0.0
    for core in range(N_CORES):
        labels, mask = host_rows[core]
        acc = res.results[core]["acc"].reshape(R).astype(np.float64)
        lnp = np.log(res.results[core]["plab"].reshape(R).astype(np.float64))
        red = acc + (1.0 - EPS) * lnp
        const = S1 + g_tab[labels]
        total += ((const - red) * mask).sum()

    loss = total / (B * (T - 1))
    out = np.float32(loss)
    if trace:
        return out, res
    return out
